# revision 5
# baseline (speedup 1.0000x reference)
"""DC_CE_Marginal_loss for Trainium2 — 8-core data-parallel Bass kernel, v2.

Layout: partition p = (b, c, d_local) — channels on the partition axis —
so every channel reduction is a TensorE matmul instead of a DVE chain.

  Launch A: per-core class-voxel counts. t arrives as [128,(b,c,d)] x 25600
      and is free-dim reduced in 2048-col pieces, alternating ScalarE
      (Copy+accum) and VectorE (tensor_reduce). Host sums 8x[128,NG] into
      global counts and builds the merge/softmax-mask operands.

  Launch B (per 512-col chunk, 4 chunks per group):
      mm1   m = W^T x            (W = I + absent-merge, PSUM bank A)
      a1    e = exp(m + bias)    (ACT, PSUM->SBUF bf16)
      mm2   S = sum_c e          (block-ones stationary; 4 chunks strip-
                                  packed into one PSUM bank, duplicated x2)
      a2    L = ln(S)            (+accum sum(lnS));  a3 ln(S+pad) (+accum)
      mm3   bank A += -lnS       (broadcast stationary, accumulate)
      a4    q = exp(m - lnS + bias)  (+accum per-partition seg)
      amr   intersect += sum(t*q);  u' += sum(t*(m-lnS))   (VectorE)

Host: sums the per-core accumulator columns in f64 and finishes
  CE = (sum lse - sum lnS - u')/NVOX, dice from (cnt, seg, intersect).
"""
import numpy as np
import ml_dtypes
import os

B, C, D, H, W = 2, 8, 64, 160, 160
NCORES = 8
DS = D // NCORES            # 8 depth slices per core
P = 128
BC = B * C                  # 16
G = DS                      # 8 groups = local depth slices
F2 = H * W                  # 25600 voxels per (b,c,d) plane
CH = 512                    # chunk columns (= one PSUM bank of f32)
NCH = F2 // CH              # 50 chunks
GRP = 4                     # chunks per S-pack group
NG = (NCH + GRP - 1) // GRP # 13 groups (12x4 + 1x2)
NGA = 13                    # launch-A count pieces (12x2048 + 1x1024)
CHA = 2048
BIG = 1e9
NVOX = B * D * H * W

# accumulator columns in launch-B output [P, NC]
SEG0 = 0
INT0 = SEG0 + NCH
U0 = INT0 + NCH
LNS0 = U0 + NCH
LSE0 = LNS0 + NG
NC = LSE0 + NG

_CACHE = {}


def _build_a():
    import concourse.bacc as bacc
    import concourse.tile as tile
    from concourse import mybir

    FA = mybir.ActivationFunctionType
    AL = mybir.AluOpType
    f32, bf16 = mybir.dt.float32, mybir.dt.bfloat16

    nc = bacc.Bacc("TRN2", num_devices=NCORES, name="loss_counts2")
    t = nc.dram_tensor("t", [BC, G, F2], bf16, kind="ExternalInput")
    out = nc.dram_tensor("cnt", [P, NGA], f32, kind="ExternalOutput")

    with tile.TileContext(nc) as tc:
        with (
            tc.tile_pool(name="tin", bufs=3) as tin,
            tc.tile_pool(name="sb", bufs=1) as sb,
        ):
            cnt = sb.tile([P, NGA], f32)
            junk = sb.tile([P, CHA], bf16)
            for i in range(NGA):
                w = min(CHA, F2 - i * CHA)
                sl = slice(i * CHA, i * CHA + w)
                t_sb = tin.tile([P, CHA], bf16, tag="t")
                nc.sync.dma_start(
                    t_sb[:, :w], t[:, :, sl].rearrange("q g f -> (q g) f"))
                if i % 2 == 0:
                    nc.vector.tensor_reduce(
                        out=cnt[:, i : i + 1], in_=t_sb[:, :w],
                        axis=mybir.AxisListType.X, op=AL.add)
                else:
                    nc.scalar.activation(
                        out=junk[:, :w], in_=t_sb[:, :w], func=FA.Copy,
                        accum_out=cnt[:, i : i + 1])
            nc.sync.dma_start(out[:], cnt[:])
    nc.compile()
    return nc


def _build_b():
    import concourse.bacc as bacc
    import concourse.tile as tile
    from concourse import mybir

    FA = mybir.ActivationFunctionType
    f32, bf16 = mybir.dt.float32, mybir.dt.bfloat16

    nc = bacc.Bacc("TRN2", num_devices=NCORES, name="loss_main2")
    x = nc.dram_tensor("x", [BC, G, F2], f32, kind="ExternalInput")
    t = nc.dram_tensor("t", [BC, G, F2], bf16, kind="ExternalInput")
    wm = nc.dram_tensor("wm", [P, P], f32, kind="ExternalInput")
    jm = nc.dram_tensor("jm", [P, 32], bf16, kind="ExternalInput")
    bm = nc.dram_tensor("bm", [P, P], f32, kind="ExternalInput")
    cl = nc.dram_tensor("cl", [P, 2], f32, kind="ExternalInput")
    out = nc.dram_tensor("out", [P, NC], f32, kind="ExternalOutput")

    with tile.TileContext(nc) as tc:
        with (
            tc.tile_pool(name="const", bufs=1) as const,
            tc.tile_pool(name="mpool", bufs=5, space="PSUM") as mpool,
            tc.tile_pool(name="spool", bufs=2, space="PSUM") as spool,
            tc.tile_pool(name="epool", bufs=3) as epool,
            tc.tile_pool(name="qpool", bufs=2) as qpool,
            tc.tile_pool(name="lpool", bufs=2) as lpool,
        ):
            wsb = const.tile([P, P], f32)
            jsb = const.tile([P, 32], bf16)
            bsb = const.tile([P, P], f32)
            csb = const.tile([P, 2], f32)
            nc.sync.dma_start(wsb[:], wm[:])
            nc.sync.dma_start(jsb[:], jm[:])
            nc.sync.dma_start(bsb[:], bm[:])
            nc.sync.dma_start(csb[:], cl[:])
            bias_col = csb[:, 0:1]
            pad_col = csb[:, 1:2]

            accs = const.tile([P, NC], f32)
            nc.vector.memset(accs[:], 0.0)
            xfull = const.tile([P, F2], f32)
            tfull = const.tile([P, F2], bf16)
            junkS = const.tile([P, CH], bf16)   # a3 output
            junkI = const.tile([P, CH], bf16)   # amr outputs
            junkU = const.tile([P, CH], bf16)

            # interleave x/t piece loads; x pieces lead since compute needs
            # them first (t only feeds the late amr stage of each group)
            for i in range(NGA):
                w = min(CHA, F2 - i * CHA)
                sl = slice(i * CHA, i * CHA + w)
                nc.sync.dma_start(
                    xfull[:, sl], x[:, :, sl].rearrange("q g f -> (q g) f"))
                nc.sync.dma_start(
                    tfull[:, sl], t[:, :, sl].rearrange("q g f -> (q g) f"))

            for gi in range(NG):
                ch0 = gi * GRP
                nch = min(GRP, NCH - ch0)
                spk = spool.tile([P, CH], f32, tag="s")
                ms = []
                for j in range(nch):
                    ch = ch0 + j
                    sl = slice(ch * CH, (ch + 1) * CH)
                    m_j = mpool.tile([P, CH], f32, tag="m")
                    nc.tensor.matmul(
                        m_j[:], wsb[:], xfull[:, sl],
                        start=True, stop=False, skip_group_check=True)
                    e_j = epool.tile([P, CH], bf16, tag="e")
                    nc.scalar.activation(
                        out=e_j[:], in_=m_j[:], func=FA.Exp,
                        bias=bias_col, scale=1.0)
                    nc.tensor.matmul(
                        spk[32 * j : 32 * j + 32, :], jsb[:], e_j[:],
                        start=True, stop=True, skip_group_check=True,
                        tile_position=(0, 32 * j))
                    ms.append((ch, sl, m_j))

                pp = 32 * nch  # populated partitions of spk
                lsb = lpool.tile([P, CH], f32, tag="l")
                nc.scalar.activation(
                    out=lsb[:pp, :], in_=spk[:pp, :], func=FA.Ln,
                    accum_out=accs[:pp, LNS0 + gi : LNS0 + gi + 1])
                nc.scalar.activation(
                    out=junkS[:pp, :], in_=spk[:pp, :], func=FA.Ln,
                    bias=pad_col[:pp], scale=1.0,
                    accum_out=accs[:pp, LSE0 + gi : LSE0 + gi + 1])

                for j, (ch, sl, m_j) in enumerate(ms):
                    nc.tensor.matmul(
                        m_j[:], bsb[32 * j : 32 * j + 16, :],
                        lsb[32 * j : 32 * j + 16, :],
                        start=False, stop=True, skip_group_check=True,
                        tile_position=(32 * j, 0))
                    q_j = qpool.tile([P, CH], bf16, tag="q")
                    nc.scalar.activation(
                        out=q_j[:], in_=m_j[:], func=FA.Exp,
                        bias=bias_col, scale=1.0,
                        accum_out=accs[:, SEG0 + ch : SEG0 + ch + 1])
                    nc.vector.affine_mul_reduce(
                        out=junkI[:],
                        accum_out=accs[:, INT0 + ch : INT0 + ch + 1],
                        in0=tfull[:, sl], in1=q_j[:], scale=1.0, bias=0.0)
                    nc.vector.affine_mul_reduce(
                        out=junkU[:],
                        accum_out=accs[:, U0 + ch : U0 + ch + 1],
                        in0=tfull[:, sl], in1=m_j[:], scale=1.0, bias=0.0)

            nc.sync.dma_start(out[:], accs[:])
    nc.compile()
    return nc


def _get(name, builder):
    if name not in _CACHE:
        _CACHE[name] = builder()
    return _CACHE[name]


def _shard_inputs(net_output, target):
    xs = np.ascontiguousarray(net_output).reshape(B, C, NCORES, G, F2)
    ts = np.ascontiguousarray(target).reshape(B, C, NCORES, G, F2)
    xmaps, tmaps = [], []
    for k in range(NCORES):
        xk = np.ascontiguousarray(xs[:, :, k]).reshape(BC, G, F2)
        tk = np.ascontiguousarray(ts[:, :, k]).reshape(BC, G, F2)
        xmaps.append(xk)
        tmaps.append(tk.astype(ml_dtypes.bfloat16))  # one-hot: exact in bf16
    return xmaps, tmaps


def _host_operands(cnt_g):
    """cnt_g [B,C] float -> (wm, jm, bm, cl, present, n)"""
    present = cnt_g > 0.5
    pm = present.astype(np.float64)
    n = pm.sum(axis=1)
    pad = n.max() - n                                   # [B]
    a = 1.0 - pm
    a[:, 0] = 0.0                                       # bg not merged into itself
    bias = pm * BIG - BIG                               # 0 present / -BIG absent

    wm = np.eye(P, dtype=np.float32)
    for b in range(B):
        for c in range(1, C):
            for g in range(G):
                wm[b * 64 + c * 8 + g, b * 64 + g] += a[b, c]

    jm = np.zeros((P, 32), dtype=np.float32)
    for b in range(B):
        for c in range(C):
            for g in range(G):
                s = b * 8 + g
                jm[b * 64 + c * 8 + g, s] = 1.0
                jm[b * 64 + c * 8 + g, 16 + s] = 1.0

    bm = np.zeros((P, P), dtype=np.float32)
    for strip in range(4):
        for b in range(B):
            for g in range(G):
                s = b * 8 + g
                for c in range(C):
                    bm[32 * strip + s, b * 64 + c * 8 + g] = -1.0

    cl = np.zeros((P, 2), dtype=np.float32)
    for b in range(B):
        for c in range(C):
            for g in range(G):
                cl[b * 64 + c * 8 + g, 0] = bias[b, c]
    for strip in range(4):
        for dup in range(2):
            for b in range(B):
                for g in range(G):
                    cl[32 * strip + 16 * dup + b * 8 + g, 1] = pad[b]

    return (wm, jm.astype(ml_dtypes.bfloat16), bm, cl, present, n)


def _run(nc, in_maps, out_name):
    if os.environ.get("K_SIM", "0") == "1":
        import concourse.bass_interp as bass_interp
        sim = bass_interp.MultiCoreSim(nc, len(in_maps))
        for k in range(len(in_maps)):
            for name, arr in in_maps[k].items():
                sim.cores[k].tensor(name)[:] = arr
        sim.simulate()
        return [{out_name: sim.cores[k].tensor(out_name).copy()}
                for k in range(len(in_maps))]
    from concourse.bass_utils import run_bass_kernel_spmd
    return run_bass_kernel_spmd(
        nc, in_maps, core_ids=list(range(len(in_maps)))).results


def run_a(tmaps):
    nc = _get("a", _build_a)
    results = _run(nc, [{"t": tk} for tk in tmaps], "cnt")
    cnt_g = np.zeros((B, C), dtype=np.float64)
    for r in results:
        # [P, NGA] -> per-partition totals -> (b, c) over g
        cnt_g += (r["cnt"].astype(np.float64).sum(axis=1)
                  .reshape(B, C, G).sum(axis=2))
    return cnt_g


def run_b(xmaps, tmaps, wm, jm, bm, cl):
    nc = _get("b", _build_b)
    in_maps = [{"x": xmaps[k], "t": tmaps[k],
                "wm": wm, "jm": jm, "bm": bm, "cl": cl}
               for k in range(NCORES)]
    results = _run(nc, in_maps, "out")
    return [r["out"].astype(np.float64) for r in results]


def _finish(cnt_g, outs, present, n):
    seg = np.zeros((B, C))
    inter = np.zeros((B, C))
    usum = 0.0
    lns = 0.0
    lse = 0.0
    for o in outs:
        seg += o[:, SEG0:SEG0 + NCH].sum(axis=1).reshape(B, C, G).sum(axis=2)
        inter += o[:, INT0:INT0 + NCH].sum(axis=1).reshape(B, C, G).sum(axis=2)
        usum += o[:, U0:U0 + NCH].sum()
        lns += o[:, LNS0:LNS0 + NG].sum() / 2.0   # strip duplication
        lse += o[:, LSE0:LSE0 + NG].sum() / 2.0
    u = usum + lns          # sum t*(m - lnS) + sum lnS = sum t*m
    ce = (lse - u) / NVOX
    dice_c = 2.0 * inter / (cnt_g + seg + 1e-5)
    dice_i = 1.0 - (present * dice_c).sum(axis=1) / n
    dc = dice_i.mean()
    return np.asarray(0.5 * ce + 0.5 * dc, dtype=np.float32)


def kernel(net_output, target):
    xmaps, tmaps = _shard_inputs(np.asarray(net_output), np.asarray(target))
    cnt_g = run_a(tmaps)
    wm, jm, bm, cl, present, n = _host_operands(cnt_g)
    outs = run_b(xmaps, tmaps, wm, jm, bm, cl)
    return _finish(cnt_g, outs, present, n)


# revision 6
# speedup vs baseline: 1.4558x; 1.4558x over previous
"""DC_CE_Marginal_loss for Trainium2 — 8-core data-parallel Bass kernel, v3.

Layout: partition p = (b, c, d_local) — channels on the partition axis —
so every channel reduction is a TensorE matmul instead of a DVE chain.

  Launch A: per-core class-voxel counts. t arrives into a persistent
      [128, 25600] tile via 13 piece-DMAs alternating the two DGE rings
      (sync HWDGE / gpsimd SWDGE); each piece is free-dim reduced,
      alternating ScalarE (Copy+accum) and VectorE (tensor_reduce). Host
      sums 8x[128,13] into global counts and builds the mask operands.

  Launch B (per 512-col chunk, 4 chunks per S-pack group):
      mm1   m = W^T x          (float32r: full-rate fp32 bits, PSUM bank A)
      a1    e = exp(m + bias)  (ACT, PSUM->SBUF bf16)
      amr   u += sum(t*m)      (VectorE, reads bank A pre-merge)
      mm2   S = sum_c e        (block-ones bf16; 4 chunks strip-packed into
                                one PSUM bank, x2 duplicated rows)
      a2    L = ln(S) -> f16;  a3  ln(S+pad) (+accum lse)   [amortized /4]
      mm3   bank A += -lnS     (f16 broadcast stationary, accumulate)
      a4    q = exp(m - lnS + bias)   (ACT)
      mm5   seg16 += sum_g q   (bf16 ones stationary, persistent PSUM bank)
      amr   intersect += sum(t*q)     (VectorE)

Host: sums per-core accumulator columns in f64; CE = (sum lse - u)/NVOX,
dice from (cnt, seg, intersect). Activation tables are pinned to the
natural_log_exp_and_others set so Exp<->Ln never reloads tables.
"""
import numpy as np
import ml_dtypes
import os

B, C, D, H, W = 2, 8, 64, 160, 160
NCORES = 8
DS = D // NCORES            # 8 depth slices per core
P = 128
BC = B * C                  # 16
G = DS                      # 8 groups = local depth slices
F2 = H * W                  # 25600 voxels per (b,c,d) plane
CH = 512                    # chunk columns (= one PSUM bank of f32)
NCH = F2 // CH              # 50 chunks
GRP = 4                     # chunks per S-pack group
NG = (NCH + GRP - 1) // GRP # 13 groups (12x4 + 1x2)
NGA = 13                    # launch-A count pieces (12x2048 + 1x1024)
CHA = 2048
BIG = 1e9
NVOX = B * D * H * W

# accumulator columns in launch-B output [P, NC]
SEGC = 0                    # 1 col, rows 0..15 = (b,c)
INT0 = 1
U0 = INT0 + NCH
LSE0 = U0 + NCH
NC = LSE0 + NG

_CACHE = {}


def _pin_act_tables():
    """Make Exp/Ln resolve only to the combined natural_log_exp_and_others
    set so the walrus/bacc table-load pass never flips tables mid-kernel."""
    import concourse.bacc as bacc  # ensure module import side effects
    from concourse.hw_specs import get_activation_tables
    from concourse import mybir
    FA = mybir.ActivationFunctionType
    tabs = get_activation_tables("gen3")  # functools.cache -> shared object
    for name, s in tabs.items():
        if name != "natural_log_exp_and_others":
            s.discard(FA.Exp)
            s.discard(FA.Ln)


def _build_a():
    import concourse.bacc as bacc
    import concourse.tile as tile
    from concourse import mybir

    _pin_act_tables()
    FA = mybir.ActivationFunctionType
    AL = mybir.AluOpType
    f32, bf16 = mybir.dt.float32, mybir.dt.bfloat16

    nc = bacc.Bacc("TRN2", num_devices=NCORES, name="loss_counts3")
    t = nc.dram_tensor("t", [BC, G, F2], bf16, kind="ExternalInput")
    out = nc.dram_tensor("cnt", [P, NGA], f32, kind="ExternalOutput")

    with tile.TileContext(nc) as tc:
        with tc.tile_pool(name="sb", bufs=1) as sb:
            tfull = sb.tile([P, F2], bf16)
            cnt = sb.tile([P, NGA], f32)
            junk = sb.tile([P, CHA], bf16)
            for i in range(NGA):
                w = min(CHA, F2 - i * CHA)
                sl = slice(i * CHA, i * CHA + w)
                eng = nc.sync if i % 2 == 0 else nc.gpsimd
                eng.dma_start(
                    tfull[:, sl], t[:, :, sl].rearrange("q g f -> (q g) f"))
                if i % 2 == 0:
                    nc.vector.tensor_reduce(
                        out=cnt[:, i : i + 1], in_=tfull[:, sl],
                        axis=mybir.AxisListType.X, op=AL.add)
                else:
                    nc.scalar.activation(
                        out=junk[:, :w], in_=tfull[:, sl], func=FA.Copy,
                        accum_out=cnt[:, i : i + 1])
            nc.sync.dma_start(out[:], cnt[:])
    nc.compile()
    return nc


def _build_b():
    import concourse.bacc as bacc
    import concourse.tile as tile
    from concourse import mybir

    _pin_act_tables()
    FA = mybir.ActivationFunctionType
    AL = mybir.AluOpType
    f32, bf16 = mybir.dt.float32, mybir.dt.bfloat16
    f32r, f16 = mybir.dt.float32r, mybir.dt.float16

    nc = bacc.Bacc("TRN2", num_devices=NCORES, name="loss_main3")
    x = nc.dram_tensor("x", [BC, G, F2], f32r, kind="ExternalInput")
    t = nc.dram_tensor("t", [BC, G, F2], bf16, kind="ExternalInput")
    wm = nc.dram_tensor("wm", [P, P], f32r, kind="ExternalInput")
    jm = nc.dram_tensor("jm", [P, 32], bf16, kind="ExternalInput")
    bm = nc.dram_tensor("bm", [P, P], f16, kind="ExternalInput")
    pm16 = nc.dram_tensor("pm16", [P, 16], bf16, kind="ExternalInput")
    cl = nc.dram_tensor("cl", [P, 2], f32, kind="ExternalInput")
    out = nc.dram_tensor("out", [P, NC], f32, kind="ExternalOutput")

    with tile.TileContext(nc) as tc:
        with (
            tc.tile_pool(name="const", bufs=1) as const,
            tc.tile_pool(name="mpool", bufs=5, space="PSUM") as mpool,
            tc.tile_pool(name="spool", bufs=2, space="PSUM") as spool,
            tc.tile_pool(name="gpool", bufs=1, space="PSUM") as gpool,
            tc.tile_pool(name="epool", bufs=4) as epool,
            tc.tile_pool(name="qpool", bufs=3) as qpool,
            tc.tile_pool(name="lpool", bufs=2) as lpool,
        ):
            wsb = const.tile([P, P], f32r)
            jsb = const.tile([P, 32], bf16)
            bsb = const.tile([P, P], f16)
            psb = const.tile([P, 16], bf16)
            csb = const.tile([P, 2], f32)
            nc.sync.dma_start(wsb[:], wm[:])
            nc.sync.dma_start(jsb[:], jm[:])
            nc.sync.dma_start(bsb[:], bm[:])
            nc.sync.dma_start(psb[:], pm16[:])
            nc.sync.dma_start(csb[:], cl[:])
            bias_col = csb[:, 0:1]
            pad_col = csb[:, 1:2]

            accs = const.tile([P, NC], f32)
            nc.vector.memset(accs[:], 0.0)
            xfull = const.tile([P, F2], f32r)
            tfull = const.tile([P, F2], bf16)
            junkS = const.tile([P, CH], bf16)   # a3 output
            junkI = const.tile([P, CH], bf16)   # amr outputs
            junkU = const.tile([P, CH], bf16)

            seg16 = gpool.tile([16, CH], f32)   # persistent seg accumulator

            # x pieces on the sync HWDGE ring, t pieces on the gpsimd ring
            for i in range(NGA):
                w = min(CHA, F2 - i * CHA)
                sl = slice(i * CHA, i * CHA + w)
                nc.sync.dma_start(
                    xfull[:, sl], x[:, :, sl].rearrange("q g f -> (q g) f"))
                nc.gpsimd.dma_start(
                    tfull[:, sl], t[:, :, sl].rearrange("q g f -> (q g) f"))

            for gi in range(NG):
                ch0 = gi * GRP
                nch = min(GRP, NCH - ch0)
                spk = spool.tile([P, CH], f32, tag="s")
                ms = []
                for j in range(nch):
                    ch = ch0 + j
                    sl = slice(ch * CH, (ch + 1) * CH)
                    m_j = mpool.tile([P, CH], f32, tag="m")
                    nc.tensor.matmul(
                        m_j[:], wsb[:], xfull[:, sl],
                        start=True, stop=False, skip_group_check=True)
                    e_j = epool.tile([P, CH], bf16, tag="e")
                    nc.scalar.activation(
                        out=e_j[:], in_=m_j[:], func=FA.Exp,
                        bias=bias_col, scale=1.0)
                    nc.vector.affine_mul_reduce(
                        out=junkU[:],
                        accum_out=accs[:, U0 + ch : U0 + ch + 1],
                        in0=tfull[:, sl], in1=m_j[:], scale=1.0, bias=0.0)
                    nc.tensor.matmul(
                        spk[32 * j : 32 * j + 32, :], jsb[:], e_j[:],
                        start=True, stop=True, skip_group_check=True,
                        tile_position=(0, 32 * j))
                    ms.append((ch, sl, m_j))

                pp = 32 * nch  # populated partitions of spk
                lsb = lpool.tile([P, CH], f16, tag="l")
                nc.scalar.activation(
                    out=lsb[:pp, :], in_=spk[:pp, :], func=FA.Ln)
                nc.scalar.activation(
                    out=junkS[:pp, :], in_=spk[:pp, :], func=FA.Ln,
                    bias=pad_col[:pp], scale=1.0,
                    accum_out=accs[:pp, LSE0 + gi : LSE0 + gi + 1])

                for j, (ch, sl, m_j) in enumerate(ms):
                    nc.tensor.matmul(
                        m_j[:], bsb[32 * j : 32 * j + 16, :],
                        lsb[32 * j : 32 * j + 16, :],
                        start=False, stop=True, skip_group_check=True,
                        tile_position=(32 * j, 0))
                    q_j = qpool.tile([P, CH], bf16, tag="q")
                    nc.scalar.activation(
                        out=q_j[:], in_=m_j[:], func=FA.Exp,
                        bias=bias_col, scale=1.0)
                    nc.tensor.matmul(
                        seg16[:, :], psb[:], q_j[:],
                        start=(ch == 0), stop=(ch == NCH - 1),
                        skip_group_check=True)
                    nc.vector.affine_mul_reduce(
                        out=junkI[:],
                        accum_out=accs[:, INT0 + ch : INT0 + ch + 1],
                        in0=tfull[:, sl], in1=q_j[:], scale=1.0, bias=0.0)

            nc.vector.tensor_reduce(
                out=accs[:16, SEGC : SEGC + 1], in_=seg16[:, :],
                axis=mybir.AxisListType.X, op=AL.add)
            nc.sync.dma_start(out[:], accs[:])
    nc.compile()
    return nc


def _get(name, builder):
    if name not in _CACHE:
        _CACHE[name] = builder()
    return _CACHE[name]


def _shard_inputs(net_output, target):
    xs = np.ascontiguousarray(net_output).reshape(B, C, NCORES, G, F2)
    ts = np.ascontiguousarray(target).reshape(B, C, NCORES, G, F2)
    xmaps, tmaps = [], []
    for k in range(NCORES):
        xk = np.ascontiguousarray(xs[:, :, k]).reshape(BC, G, F2)
        tk = np.ascontiguousarray(ts[:, :, k]).reshape(BC, G, F2)
        xmaps.append(xk)
        tmaps.append(tk.astype(ml_dtypes.bfloat16))  # one-hot: exact in bf16
    return xmaps, tmaps


def _host_operands(cnt_g):
    """cnt_g [B,C] float -> (wm, jm, bm, p16, cl, present, n)"""
    present = cnt_g > 0.5
    pm = present.astype(np.float64)
    n = pm.sum(axis=1)
    pad = n.max() - n                                   # [B]
    a = 1.0 - pm
    a[:, 0] = 0.0                                       # bg not merged into itself
    bias = pm * BIG - BIG                               # 0 present / -BIG absent

    wm = np.eye(P, dtype=np.float32)
    for b in range(B):
        for c in range(1, C):
            for g in range(G):
                wm[b * 64 + c * 8 + g, b * 64 + g] += a[b, c]

    jm = np.zeros((P, 32), dtype=np.float32)
    for b in range(B):
        for c in range(C):
            for g in range(G):
                s = b * 8 + g
                jm[b * 64 + c * 8 + g, s] = 1.0
                jm[b * 64 + c * 8 + g, 16 + s] = 1.0

    bm = np.zeros((P, P), dtype=np.float32)
    for strip in range(4):
        for b in range(B):
            for g in range(G):
                s = b * 8 + g
                for c in range(C):
                    bm[32 * strip + s, b * 64 + c * 8 + g] = -1.0

    p16 = np.zeros((P, 16), dtype=np.float32)
    for b in range(B):
        for c in range(C):
            for g in range(G):
                p16[b * 64 + c * 8 + g, b * 8 + c] = 1.0

    cl = np.zeros((P, 2), dtype=np.float32)
    for b in range(B):
        for c in range(C):
            for g in range(G):
                cl[b * 64 + c * 8 + g, 0] = bias[b, c]
    for strip in range(4):
        for dup in range(2):
            for b in range(B):
                for g in range(G):
                    cl[32 * strip + 16 * dup + b * 8 + g, 1] = pad[b]

    return (wm, jm.astype(ml_dtypes.bfloat16), bm.astype(np.float16),
            p16.astype(ml_dtypes.bfloat16), cl, present, n)


def _run(nc, in_maps, out_name):
    if os.environ.get("K_SIM", "0") == "1":
        import concourse.bass_interp as bass_interp
        sim = bass_interp.MultiCoreSim(nc, len(in_maps))
        for k in range(len(in_maps)):
            for name, arr in in_maps[k].items():
                sim.cores[k].tensor(name)[:] = arr
        sim.simulate()
        return [{out_name: sim.cores[k].tensor(out_name).copy()}
                for k in range(len(in_maps))]
    from concourse.bass_utils import run_bass_kernel_spmd
    return run_bass_kernel_spmd(
        nc, in_maps, core_ids=list(range(len(in_maps)))).results


def run_a(tmaps):
    nc = _get("a", _build_a)
    results = _run(nc, [{"t": tk} for tk in tmaps], "cnt")
    cnt_g = np.zeros((B, C), dtype=np.float64)
    for r in results:
        cnt_g += (r["cnt"].astype(np.float64).sum(axis=1)
                  .reshape(B, C, G).sum(axis=2))
    return cnt_g


def run_b(xmaps, tmaps, wm, jm, bm, p16, cl):
    nc = _get("b", _build_b)
    in_maps = [{"x": xmaps[k], "t": tmaps[k],
                "wm": wm, "jm": jm, "bm": bm, "pm16": p16, "cl": cl}
               for k in range(NCORES)]
    results = _run(nc, in_maps, "out")
    return [r["out"].astype(np.float64) for r in results]


def _finish(cnt_g, outs, present, n):
    seg = np.zeros((B, C))
    inter = np.zeros((B, C))
    usum = 0.0
    lse = 0.0
    for o in outs:
        seg += o[:16, SEGC].reshape(B, C)
        inter += o[:, INT0:INT0 + NCH].sum(axis=1).reshape(B, C, G).sum(axis=2)
        usum += o[:, U0:U0 + NCH].sum()
        lse += o[:, LSE0:LSE0 + NG].sum() / 2.0   # strip duplication
    ce = (lse - usum) / NVOX
    dice_c = 2.0 * inter / (cnt_g + seg + 1e-5)
    dice_i = 1.0 - (present * dice_c).sum(axis=1) / n
    dc = dice_i.mean()
    return np.asarray(0.5 * ce + 0.5 * dc, dtype=np.float32)


def kernel(net_output, target):
    xmaps, tmaps = _shard_inputs(np.asarray(net_output), np.asarray(target))
    cnt_g = run_a(tmaps)
    wm, jm, bm, p16, cl, present, n = _host_operands(cnt_g)
    outs = run_b(xmaps, tmaps, wm, jm, bm, p16, cl)
    return _finish(cnt_g, outs, present, n)
